# revision 23
# baseline (speedup 1.0000x reference)
"""Trainium2 Bass kernel for a 2-layer CIN (Compressed Interaction Network).

Reference computation (per batch b, embedding dim d):
    h1[q] = sum_{f,g} x[f] x[g] W0[q, f*39+g]          (f,g in 0..38)
    h2[h] = sum_{f,q} x[f] h1[q] W1[h, f*128+q]        (f in 0..38, q in 0..127)
    out[b] = concat(sum_d h1, sum_d h2)                 -> [B, 256]

Device mapping (data-parallel over batch across 8 cores, 256 b's each):
  * Layer 1 via polarization ("sum of squares"):  x_i x_j =
    ((x_i+x_j)^2 - x_i^2 - x_j^2)/2.  With 780 fixed linear forms V (39
    singles + 741 pair sums, padded to 896 = 7*128) and re-packed
    coefficients C:  h1 = C^T (V^T x)^2.  The C contraction directly
    produces the TRANSPOSED h1t[(b,d), q] (stationary = ysq chunk blocks),
    so no PE transpose pass is needed.
  * Layer 2 exploits  sum_d h2[b,:,d] = W1flat @ vec(S_b),
    S_b[f,q] = sum_d x[b,f,d] h1[b,q,d]:  S^T chunks come from k=128
    matmuls of h1t against a host-packed block-diagonal transposed-x
    operand (8 batches, 8*40 cols; the 40th column of each batch block is
    all-ones over d, so the same matmul also emits sum_d h1 = out1 free).
    Final contraction: 2 x 39 k=128 matmuls against W1.

All DRAM tensors are packed partition-major on the host so every DMA is a
single transfer with 128 fat contiguous descriptors, all on the SP HWDGE
queue (ACT-queue DMAs measured pathologically slow; many small DMAs pay a
~4-5 us fixed cost each).  Squares are split ACT/DVE; PSUM: 4 single-chunk
y banks + 2 h1t banks + 2 S^T banks.
"""

import numpy as np

import concourse.mybir as mybir
import concourse.tile as tile
from concourse import bacc
from concourse.bass import ts
from concourse.bass_utils import run_bass_kernel_spmd

B, F0, D = 2048, 39, 16
H1, H2 = 128, 128
NCORES = 8
BC = B // NCORES          # 256 batches per core
BT = 32                   # batches per tile (round)
NT = BC // BT             # 8 tiles per core
N = BT * D                # 512 columns per tile (cols = (b, d), d inner)
NFP = 896                 # forms padded to 7*128
NCHUNK = 7
CW = 128                  # forms per chunk
SB = 8                    # batches per S-chunk (8 b x 16 d = 128 partitions)
NSC = BT // SB            # 4 S-chunks per tile
FW = F0 + 1               # 39 f cols + 1 ones col (-> sum_d h1)
SW = SB * FW              # 320 cols per S-chunk

F16 = mybir.dt.float16
F32 = mybir.dt.float32
WPK = 2 * NFP + F0 * H2   # wpack free size: vp | cp | w1p


def pack_weights(W0: np.ndarray, W1: np.ndarray):
    """Host-side repack of CIN weights into one device tensor (fp16)."""
    W0m = W0[:, :, 0].reshape(H1, F0, F0).astype(np.float64)
    W1m = W1[:, :, 0].reshape(H2, F0, H1).astype(np.float64)

    V = np.zeros((128, NFP), dtype=np.float64)   # k-padded: rows 39.. = 0
    C = np.zeros((NFP, H1), dtype=np.float64)
    for i in range(F0):
        V[i, i] = 1.0
        Bi = W0m[:, i, :] + W0m[:, :, i]          # [H, F]
        C[i, :] = W0m[:, i, i] - 0.5 * (Bi.sum(axis=1) - 2.0 * W0m[:, i, i])
    k = F0
    for i in range(F0):
        for j in range(i + 1, F0):
            V[i, k] = 1.0
            V[j, k] = 1.0
            C[k, :] = 0.5 * (W0m[:, i, j] + W0m[:, j, i])
            k += 1
    c_pack = C.reshape(NCHUNK, CW, H1).transpose(1, 0, 2)   # [128, 7, 128]

    w1p = W1m.transpose(2, 1, 0)                   # [q=128, f=39, h=128]

    wpack = np.concatenate([
        V.astype(np.float16).reshape(128, NFP),
        c_pack.astype(np.float16).reshape(128, NFP),
        w1p.astype(np.float16).reshape(128, F0 * H2),
    ], axis=1)
    return {"wpack": np.ascontiguousarray(wpack)}


def pack_x(x_core: np.ndarray):
    """Per-core input repack: f-major padded x + block-diagonal transposed x.

    x_core: [BC, 39, 16] float.  Returns (both partition-major contiguous)
      xp  [128, BC, D] fp16, xp[f, b, d] = x[b, f, d] (rows 39.. zero);
      xt4 [128, NT, NSC, SW] fp16: chunk (t, c) covers batches 32t+8c+j,
          partition 16j+d, col 40j+f, value x[b, f, d]; col 40j+39 is 1.0
          on partitions 16j..16j+15 (emits sum_d h1 through the S matmul).
    """
    x16 = x_core.astype(np.float16)
    xp = np.zeros((128, BC, D), dtype=np.float16)
    xp[:F0] = x16.transpose(1, 0, 2)
    xt4 = np.zeros((128, NT, NSC, SW), dtype=np.float16)
    x6 = x16.reshape(NT, NSC, SB, F0, D)
    for j in range(SB):
        xt4[D * j:D * (j + 1), :, :, FW * j:FW * j + F0] = (
            x6[:, :, j].transpose(3, 0, 1, 2))
        xt4[D * j:D * (j + 1), :, :, FW * j + F0] = 1.0
    return xp, np.ascontiguousarray(xt4)


def build(reps: int = 1, unroll: bool = False):
    """Build the per-core Bass module. reps>1 wraps the body in a HW loop
    (wall-clock timing only -- the graded path uses reps=1)."""
    nc = bacc.Bacc("TRN2", target_bir_lowering=False, debug=False,
                   num_devices=NCORES)

    x_h = nc.dram_tensor("xp", [128, BC, D], F16, kind="ExternalInput")
    xt4_h = nc.dram_tensor("xt4", [128, NT, NSC, SW], F16,
                           kind="ExternalInput")
    # wpack = [vp | cp | w1p] along the free dim, partition-major
    wp_h = nc.dram_tensor("wpack", [128, WPK], F16, kind="ExternalInput")
    # out[p][hf][ly][b']: layer ly output for batch hf*128+b', unit p
    out_h = nc.dram_tensor("out", [128, 2, 2, 128], F16, kind="ExternalOutput")

    with tile.TileContext(nc) as tc:
        with (
            tc.tile_pool(name="wpool", bufs=2) as wpool,
            tc.tile_pool(name="sallp", bufs=2) as sallp,
            tc.tile_pool(name="xpool", bufs=2) as xpool,
            tc.tile_pool(name="xtp", bufs=2) as xtp,
            tc.tile_pool(name="ysq", bufs=3) as ysqp,
            tc.tile_pool(name="ystg", bufs=2) as ystgp,
            tc.tile_pool(name="h1t", bufs=2) as h1tp,
            tc.tile_pool(name="outp", bufs=2) as outp,
            tc.tile_pool(name="ps", space="PSUM", bufs=1) as ps,
        ):
            def body(pending_out=None):
                wp_sb = wpool.tile([128, WPK], F16, tag="wp", name="wp")
                v_sb = wp_sb[:, :NFP]
                c_sb = wp_sb[:, NFP:2 * NFP].rearrange(
                    "p (c h) -> p c h", c=NCHUNK)
                w1_sb = wp_sb[:, 2 * NFP:].rearrange(
                    "p (f h) -> p f h", f=F0)
                # S^T: [q, b, f|sum]
                sall_sb = sallp.tile([128, BC, FW], F16, tag="sall",
                                     name="sall")
                oh = outp.tile([128, 2, 2, 128], F16, tag="oh", name="oh")
                x_all = xpool.tile([128, BC, D], F16, tag="x", name="x_all")
                xt_all = xtp.tile([128, NT, NSC, SW], F16, tag="xt",
                                  name="xt_all")
                nc.sync.dma_start(out=wp_sb[:, :2 * NFP],
                                  in_=wp_h.ap()[:, :2 * NFP])
                nc.sync.dma_start(out=x_all[:], in_=x_h.ap())
                nc.sync.dma_start(out=xt_all[:], in_=xt4_h.ap())
                nc.sync.dma_start(out=wp_sb[:, 2 * NFP:],
                                  in_=wp_h.ap()[:, 2 * NFP:])
                if pending_out is not None:
                    # previous logical rep's store, emitted after this rep's
                    # load triggers so the SP DGE FIFO can prefetch these
                    # loads during the previous rep's compute
                    nc.sync.dma_start(out=out_h.ap(), in_=pending_out[:])

                xs = [x_all[:, ts(r, BT), :] for r in range(NT)]
                xts = [xt_all[:, r] for r in range(NT)]
                ysqs, ystgs, h1tps, h1ts = {}, {}, {}, {}

                def final_half(hf):
                    # out2 for b in [hf*128, ..+128); out1 half riding along
                    out2_ps = ps.tile([128, N], F32, tag="y", bufs=4,
                                      name=f"o2_{hf}")
                    for f in range(F0):
                        nc.tensor.matmul(out2_ps[:, :128], w1_sb[:, f, :],
                                         sall_sb[:, ts(hf, 128), f],
                                         start=(f == 0), stop=(f == F0 - 1))
                    nc.scalar.copy(oh[:, hf, 1, :], out2_ps[:, :128])
                    nc.vector.tensor_copy(out=oh[:, hf, 0, :],
                                          in_=sall_sb[:, ts(hf, 128), F0])

                def vsq(r, u):
                    # V chunk u -> y PSUM; square -> ysq SBUF fp16.
                    # ACT is the only engine that can square straight out of
                    # PSUM; for chunks 3 and 6 DVE stages a PSUM->SBUF copy
                    # and the (otherwise idle) GPSIMD engine squares it.
                    y_ps = ps.tile([128, N], F32, tag="y", bufs=4,
                                   name=f"y_{r}_{u}")
                    nc.tensor.matmul(y_ps[:], v_sb[:, ts(u, CW)], xs[r],
                                     start=True, stop=True)
                    dst = ysqs[r][:, u, :]
                    if u in (3, 6):
                        stg = ystgs[r][:, 0 if u == 3 else 1, :]
                        nc.vector.tensor_copy(out=stg, in_=y_ps[:])
                        nc.gpsimd.tensor_mul(out=dst, in0=stg, in1=stg)
                    else:
                        nc.scalar.square(dst, y_ps[:])

                def cgrp(r, g):
                    # h1t[(b,d) block g, q] += ysq_j^T c_j  over all chunks j
                    for j in range(NCHUNK):
                        nc.tensor.matmul(h1tps[r][:, g, :],
                                         ysqs[r][:, j, ts(g, 128)],
                                         c_sb[:, j, :],
                                         start=(j == 0), stop=(j == NCHUNK - 1))

                def smm(t, u):
                    # S^T chunk for batches 32t+8u.. ; copy into sall
                    st_ps = ps.tile([128, 512], F32, tag="st", bufs=2,
                                    name=f"st_{t}_{u}")
                    nc.tensor.matmul(st_ps[:, :SW], h1ts[t][:, u, :],
                                     xts[t][:, u, :], start=True, stop=True)
                    b0 = BT * t + SB * u
                    src = st_ps[:, :SW].rearrange("p (j f) -> p j f", f=FW)
                    dst = sall_sb[:, b0:b0 + SB, :]
                    # u==1 on ACT in steady state; tail rounds (no squares
                    # left on ACT) take more chunks on ACT
                    if (u == 1) or (t >= NT - 3 and u in (0, 2)):
                        nc.scalar.copy(dst, src)
                    else:
                        nc.vector.tensor_copy(out=dst, in_=src)

                # lags: square(r) same round as V(r); C'(r-2) two rounds
                # later (full-round slack for the last squares); S(r-3).
                if True:
                    for r in range(NT + 3):
                        if r < NT:
                            ysqs[r] = ysqp.tile([128, NCHUNK, N], F16,
                                                tag="ysq", name=f"ysq_{r}")
                            ystgs[r] = ystgp.tile([128, 2, N], F16,
                                                  tag="ystg", name=f"ystg_{r}")
                        if 2 <= r < NT + 2:
                            h1tps[r - 2] = ps.tile([128, NSC, 128], F32,
                                                   tag="h1t_ps", bufs=2,
                                                   name=f"h1tp_{r - 2}")

                        # PE order: S(t) chunks interleaved among V chunks
                        # and C' groups so every PSUM handoff has slack.
                        for g in range(NSC):
                            if 3 <= r:
                                smm(r - 3, g)
                            if r < NT:
                                vsq(r, g)
                            if 2 <= r < NT + 2:
                                cgrp(r - 2, g)
                        if r < NT:
                            vsq(r, 4)
                            vsq(r, 5)
                            vsq(r, 6)
                        # h1t' -> SBUF fp16 (stationary for S two rounds on)
                        if 2 <= r < NT + 2:
                            h1ts[r - 2] = h1tp.tile([128, NSC, 128], F16,
                                                    tag="h1t",
                                                    name=f"h1t_{r - 2}")
                            nc.vector.tensor_copy(out=h1ts[r - 2][:],
                                                  in_=h1tps[r - 2][:])

                        if r == 7:
                            final_half(0)   # b 0..127: S tiles 0..3 done

                    final_half(1)
                return oh

            if reps == 1:
                oh = body()
                nc.sync.dma_start(out=out_h.ap(), in_=oh[:])
            elif unroll:
                pend = None
                for _ in range(reps):
                    pend = body(pend)
                nc.sync.dma_start(out=out_h.ap(), in_=pend[:])
            else:
                # two logical reps per HW-loop iteration so the tile pools
                # double-buffer across reps (buffers do NOT rotate across
                # For_i iterations) and loads prefetch during compute.
                assert reps % 2 == 0, "timing reps must be even"
                with tc.For_i(0, reps // 2):
                    oh_a = body()
                    oh_b = body(oh_a)
                nc.sync.dma_start(out=out_h.ap(), in_=oh_b[:])

    nc.compile()
    return nc


_CACHE: dict = {}


def _get_module(reps: int = 1):
    if reps not in _CACHE:
        _CACHE[reps] = build(reps)
    return _CACHE[reps]


def run(input: np.ndarray, W0: np.ndarray, W1: np.ndarray, reps: int = 1):
    nc = _get_module(reps)
    packs = pack_weights(np.asarray(W0), np.asarray(W1))
    x_np = np.asarray(input)
    in_maps = []
    for c in range(NCORES):
        xp, xt4 = pack_x(x_np[c * BC:(c + 1) * BC])
        m = {"xp": xp, "xt4": xt4}
        m.update(packs)
        in_maps.append(m)
    res = run_bass_kernel_spmd(nc, in_maps, core_ids=list(range(NCORES)))
    out = np.empty((B, 256), dtype=np.float32)
    for c in range(NCORES):
        o = res.results[c]["out"]          # [128, 2, 2, 128] fp16
        for hf in range(2):
            sl = slice(c * BC + hf * 128, c * BC + hf * 128 + 128)
            out[sl, :128] = o[:, hf, 0, :].T
            out[sl, 128:] = o[:, hf, 1, :].T
    return out


def kernel(input: np.ndarray, W0: np.ndarray, W1: np.ndarray) -> np.ndarray:
    return run(input, W0, W1, reps=1)
